# revision 54
# baseline (speedup 1.0000x reference)
"""Trainium2 Bass kernel for nn_BertClassifier_77309411685.

Data-parallel over 8 NeuronCores: each core handles 256 samples of the
2048-sample batch; the small base linear and 12 expert heads are replicated.

Per-core device algorithm (B_c=256 samples, processed as 2 halves of 128):
  1. indirect-DMA gather: for each sample, the 8 consecutive embedding rows
     starting at `start` (spans are 1..8 long and start <= S-9, so 8 rows are
     always in bounds), split into two 4-row chunks so the masked-mean can
     start while the second chunk is still in flight. One sample per
     partition.
  2. masked mean over the span via fused DVE multiply-accumulate with
     per-partition weights (i < len) / len.
  3. two static context rows loaded with strided DMA.
  4. PE transposes build featT [3H=2304, 256] (feature-major) from the
     sample-major center/context tiles; 3 transposes share one PSUM bank and
     drain with a single strided copy.
  5. base linear: hiddenT[inner, b] = relu(W_base @ feat + b_base) via 18
     K-chunk matmuls per 128-wide inner tile, bias+relu fused in the
     PSUM->SBUF activation.
  6. expert heads: compute all 12 experts at once, out36[b, e*3+n], with the
     bias folded in as an extra K=1 matmul against a ones row; then select
     the right expert per sample with an is_equal mask and a strided reduce.

Constants (identity matrix, iota ramps, per-partition row bases) are shipped
as one small DRAM input instead of being built with gpsimd ops on device.
"""

import numpy as np
from contextlib import ExitStack

import concourse.bass as bass
import concourse.tile as tile
from concourse import bacc, mybir
from concourse.bass import IndirectOffsetOnAxis
from concourse.bass_utils import run_bass_kernel_spmd

F32 = mybir.dt.float32
I32 = mybir.dt.int32

B, S, H = 2048, 256, 768
INNER, NB_CTX, NB_EXPERTS, NB_LABELS = 256, 2, 12, 3
NCORES = 8
BC = B // NCORES           # 256 samples per core
F3H = (NB_CTX + 1) * H     # 2304
KC = F3H // 128            # 18 contraction chunks
NE = NB_EXPERTS * NB_LABELS  # 36
SPAN = 8                   # max span length; always safe to gather 8 rows
HC = H // 128              # 6 h-chunks per feature block

# The reference picks 2 static context positions host-side with this exact rng.
CTX_IDX = [int(v) for v in np.random.default_rng(seed=0).choice(np.arange(S), size=NB_CTX)]

# Dtype used for the PE matmul operands (featT, weights, hiddenT).
#  float32: 4 cycles/row (2 half-speed passes) — exact baseline.
#  float32r: 1 cycle/row at N>=256 — reduced-mantissa single pass.
#  bfloat16: 1 cycle/row + fast weight load, half the SBUF traffic.
# Accumulation is always fp32 in PSUM; center/ctx stay fp32 until the single
# rounding point at the PSUM->featT copy.
MM_DT = mybir.dt.float32r
# Dtype the span gather lands in SBUF as (the DMA casts in flight when bf16 —
# halves the SBUF-port bytes of the SWDGE stream; measured no win, keep f32).
GATHER_DT = mybir.dt.float32

# Const blob layout (f32 columns): identity [0:128), io8f [128:136),
# io36f [136:172).  Separate int32 blob: rowbase [p, 0] = p*S.
C_ID, C_IO8, C_IO36, C_NF = 0, 128, 136, 172


def _build():
    nc = bacc.Bacc(
        "TRN2",
        target_bir_lowering=False,
        debug=False,
        enable_asserts=False,
        num_devices=NCORES,
    )
    emb = nc.dram_tensor("emb", [BC * S, H], F32, kind="ExternalInput").ap()
    pos = nc.dram_tensor("pos", [BC, 2], I32, kind="ExternalInput").ap()
    cat = nc.dram_tensor("cat", [BC, 1], I32, kind="ExternalInput").ap()
    wbT = nc.dram_tensor("wbT", [F3H, INNER], F32, kind="ExternalInput").ap()
    bb = nc.dram_tensor("bb", [INNER], F32, kind="ExternalInput").ap()
    wexpT = nc.dram_tensor("wexpT", [INNER + 1, NE], F32, kind="ExternalInput").ap()
    cstf = nc.dram_tensor("cstf", [128, C_NF], F32, kind="ExternalInput").ap()
    onesd = nc.dram_tensor("onesd", [1, 256], F32, kind="ExternalInput").ap()
    # Host-computed gather row indices, col h*3+ci: chunk start rows, with the
    # skip marker (BIG, beyond the bounds check) baked in for chunks 1/2 when
    # the span doesn't reach them.
    gidx = nc.dram_tensor("gidx", [128, 6], I32, kind="ExternalInput").ap()
    out = nc.dram_tensor("out", [BC, NB_LABELS], F32, kind="ExternalOutput").ap()

    emb3d = emb.rearrange("(b s) h -> b s h", s=S)
    TD = MM_DT
    GDT = GATHER_DT

    def asTD(dram_ap):
        # f32 DRAM bits reinterpreted as the PE dtype (f32r shares the layout).
        return dram_ap.bitcast(TD) if TD == mybir.dt.float32r else dram_ap

    with tile.TileContext(nc) as tc, ExitStack() as ctx:
        pool = ctx.enter_context(tc.tile_pool(name="main", bufs=1))
        gpool = ctx.enter_context(tc.tile_pool(name="gp", bufs=2))
        spool = ctx.enter_context(tc.tile_pool(name="small", bufs=2))
        pst = ctx.enter_context(tc.tile_pool(name="pst", bufs=2, space="PSUM"))
        psh = ctx.enter_context(tc.tile_pool(name="psh", bufs=2, space="PSUM"))
        ps36p = ctx.enter_context(tc.tile_pool(name="ps36p", bufs=2, space="PSUM"))

        # --- phase 0: tiny front-of-queue loads the gather depends on ---
        gidx_t = pool.tile([128, 6], I32)
        nc.sync.dma_start(gidx_t[:], gidx[:, :])
        pos_t = pool.tile([128, 4], I32)  # [p, h*2 + j] = pos[h*128+p, j]
        nc.sync.dma_start(pos_t[:].rearrange("p (h j) -> p h j", j=2),
                          pos.rearrange("(h p) j -> p h j", p=128))
        cstf_t = pool.tile([128, C_NF], F32)
        nc.sync.dma_start(cstf_t[:], cstf[:, :])
        cat_t = pool.tile([128, 2], I32)  # [p, h] = cat[h*128+p]
        nc.sync.dma_start(cat_t[:].rearrange("p (h j) -> p h j", j=1),
                          cat.rearrange("(h p) j -> p h j", p=128))

        io8f = cstf_t[:, C_IO8:C_IO8 + SPAN]
        io36f = cstf_t[:, C_IO36:C_IO36 + NE]
        # Identity for PE transposes, in the PE dtype (separate tile so the
        # fp32r verifier sees a rounded producer).
        id_t = pool.tile([128, 128], TD)
        nc.sync.dma_start(id_t[:], asTD(cstf[:, C_ID:C_ID + 128]))
        identity = id_t[:]

        # Pre-zero the conditional gather chunks first thing on DVE: skipped
        # samples keep zeros, and the zero span weights keep them out of the
        # mean. Must land before the conditional gathers start writing.
        gz = []
        for ci in (1, 2):
            for h in range(2):
                g = gpool.tile([128, 2 * H], GDT, tag=f"g{h}{ci}", bufs=1)
                nc.vector.memset(g[:], 0.0)
                gz.append(g)

        # --- phase 1: per-half index chains + gathers, earliest possible ---
        # Row chunks per sample: [0:4), [4:6), [6:8).  Chunks 1/2 are skipped
        # per-sample via the DGE bounds check when the span doesn't reach them
        # (len<=4 / len<=6); their tiles are pre-zeroed so skipped lanes stay 0
        # and the zero weights keep them out of the mean.
        g_chunks = [[None, None, None], [None, None, None]]
        # chunk 0 (always needed) goes out as soon as its indices land.
        # The two halves ride different SWDGE queue rows so each SDMA engine
        # round-robins between two descriptor streams (hides HBM read latency).
        for h in range(2):
            g0 = gpool.tile([128, 4 * H], GDT, tag=f"g{h}0", bufs=1)
            nc.gpsimd.indirect_dma_start(
                out=g0[:], out_offset=None, in_=emb,
                in_offset=IndirectOffsetOnAxis(ap=gidx_t[:, 3 * h:3 * h + 1], axis=0),
            )
            g_chunks[h][0] = g0

        # conditional chunks: gather with the bounds check dropping the
        # per-sample skip-marked indices; interleave halves so each half's
        # last chunk lands as early as possible
        for ci in (1, 2):
            for h in range(2):
                g = gz[(ci - 1) * 2 + h]
                nc.gpsimd.indirect_dma_start(
                    out=g[:], out_offset=None, in_=emb,
                    in_offset=IndirectOffsetOnAxis(
                        ap=gidx_t[:, 3 * h + ci:3 * h + ci + 1], axis=0),
                    bounds_check=BC * S - 1, oob_is_err=False,
                )
                g_chunks[h][ci] = g

        w8_h = []
        for h in range(2):
            # span weights w8[p, i] = (i < len) / len
            len_i = spool.tile([128, 1], I32, tag=f"leni{h}", bufs=1)
            nc.vector.tensor_tensor(out=len_i[:], in0=pos_t[:, 2 * h + 1:2 * h + 2],
                                    in1=pos_t[:, 2 * h:2 * h + 1],
                                    op=mybir.AluOpType.subtract)
            len_f = spool.tile([128, 1], F32, tag=f"lenf{h}", bufs=1)
            nc.vector.tensor_copy(len_f[:], len_i[:])
            rcp = spool.tile([128, 1], F32, tag=f"rcp{h}", bufs=1)
            nc.vector.reciprocal(rcp[:], len_f[:])
            w8 = spool.tile([128, SPAN], F32, tag=f"w8{h}", bufs=1)
            nc.vector.tensor_scalar(w8[:], io8f, len_f[:, :1], rcp[:, :1],
                                    op0=mybir.AluOpType.is_lt,
                                    op1=mybir.AluOpType.mult)
            w8_h.append(w8)

        # --- phase 2: replicated weights first (the ctx-chunk matmuls need
        # them mid-gather), then context rows ---
        # wbT is shipped pre-laid-out: wbT_host[p, c*INNER+m] = W_base[m, c*128+p].
        # Split into 6 medium DMAs so the packets interleave gently with the
        # concurrent indirect gathers.
        wbT_t = pool.tile([128, KC * INNER], TD)
        wbT_c = wbT.rearrange("(p x) m -> p (x m)", p=128)
        step = KC * INNER // 6
        for j in range(6):
            sl = slice(j * step, (j + 1) * step)
            if TD == mybir.dt.bfloat16:
                nc.gpsimd.dma_start(wbT_t[:, sl], wbT_c[:, sl])
            else:
                nc.sync.dma_start(wbT_t[:, sl], asTD(wbT_c[:, sl]))
        bb_t = pool.tile([128, 2], F32)  # bb_t[p, t] = b_base[t*128 + p]
        nc.sync.dma_start(bb_t[:], bb.rearrange("(t p) -> p t", p=128))
        wexpA = pool.tile([128, NE], TD)
        wexpB = pool.tile([128, NE], TD)
        wexpC = pool.tile([1, NE], TD)
        if TD == mybir.dt.bfloat16:
            nc.gpsimd.dma_start(wexpA[:], wexpT[0:128, :])
            nc.gpsimd.dma_start(wexpB[:], wexpT[128:256, :])
            nc.gpsimd.dma_start(wexpC[:], wexpT[256:257, :])
        else:
            nc.sync.dma_start(wexpA[:], asTD(wexpT[0:128, :]))
            nc.sync.dma_start(wexpB[:], asTD(wexpT[128:256, :]))
            nc.sync.dma_start(wexpC[:], asTD(wexpT[256:257, :]))
        ones1 = pool.tile([1, 256], TD)
        if TD == mybir.dt.float32r:
            nc.sync.dma_start(ones1[:], asTD(onesd[:, :]))
        else:
            nc.vector.memset(ones1[:], 1.0)

        ctxs = []
        for h in range(2):
            b0 = h * 128
            ctx0 = gpool.tile([128, H], TD, tag=f"ctx0{h}", bufs=1)
            nc.sync.dma_start(ctx0[:], asTD(emb3d[b0:b0 + 128, CTX_IDX[0], :]))
            ctx1 = gpool.tile([128, H], TD, tag=f"ctx1{h}", bufs=1)
            nc.sync.dma_start(ctx1[:], asTD(emb3d[b0:b0 + 128, CTX_IDX[1], :]))
            ctxs.append((ctx0, ctx1))

        # --- phase 3a: ctx transposes + copies (their data lands early) ---
        featT = pool.tile([128, KC * 256], TD)
        featT3 = featT[:].rearrange("p (si rest) -> p si rest", si=3)
        for h in range(2):
            ctx0, ctx1 = ctxs[h]
            for c in range(HC):
                tpc = pst.tile([128, 2 * 128], TD, tag="tpc")
                for si, src in enumerate((ctx0, ctx1)):
                    nc.tensor.transpose(tpc[:, si * 128:(si + 1) * 128],
                                        src[:, c * 128:(c + 1) * 128], identity)
                col = c * 256 + h * 128
                nc.scalar.copy(featT3[:, 1:3, col:col + 128],
                               tpc[:].rearrange("p (si x) -> p si x", si=2))

        # --- phase 3b: ctx part of the base linear runs during the gather ---
        hiddenT = pool.tile([128, 2 * 256], TD)
        accs = [psh.tile([128, 256], F32, tag=f"acc{mt}", bufs=1, name=f"acc{mt}")
                for mt in range(2)]
        for c in range(HC, KC):
            for mt in range(2):
                nc.tensor.matmul(
                    accs[mt][:],
                    lhsT=wbT_t[:, c * INNER + mt * 128: c * INNER + (mt + 1) * 128],
                    rhs=featT[:, c * 256:(c + 1) * 256],
                    start=(c == HC), stop=False,
                )

        # --- phase 3c: masked mean + center transposes ---
        catf_h = []
        for h in range(2):
            w8 = w8_h[h]
            # accA: rows 0-3 (chunk 0 lands first)
            accA = gpool.tile([128, H], F32, tag=f"accA{h}", bufs=1)
            nc.vector.tensor_scalar(accA[:], g_chunks[h][0][:, 0:H], w8[:, 0:1],
                                    None, op0=mybir.AluOpType.mult)
            for i in range(1, 4):
                off = i * H
                nc.vector.scalar_tensor_tensor(
                    out=accA[:], in0=g_chunks[h][0][:, off:off + H],
                    scalar=w8[:, i:i + 1], in1=accA[:],
                    op0=mybir.AluOpType.mult, op1=mybir.AluOpType.add)
            # accB: rows 4-7 (conditional chunks land last)
            accB = gpool.tile([128, H], F32, tag=f"accB{h}", bufs=1)
            nc.vector.tensor_scalar(accB[:], g_chunks[h][1][:, 0:H], w8[:, 4:5],
                                    None, op0=mybir.AluOpType.mult)
            for i, (ci, off) in enumerate([(1, H), (2, 0), (2, H)], start=5):
                nc.vector.scalar_tensor_tensor(
                    out=accB[:], in0=g_chunks[h][ci][:, off:off + H],
                    scalar=w8[:, i:i + 1], in1=accB[:],
                    op0=mybir.AluOpType.mult, op1=mybir.AluOpType.add)
            center = gpool.tile([128, H], TD, tag=f"center{h}", bufs=1)
            nc.vector.tensor_tensor(out=center[:], in0=accA[:], in1=accB[:],
                                    op=mybir.AluOpType.add)

            # center transposes; one ACT copy per h-chunk keeps DVE free
            for c in range(HC):
                tp = pst.tile([128, 128], TD, tag="tp")
                nc.tensor.transpose(tp[:], center[:, c * 128:(c + 1) * 128],
                                    identity)
                col = c * 256 + h * 128
                nc.scalar.copy(featT3[:, 0:1, col:col + 128],
                               tp[:].rearrange("p (si x) -> p si x", si=1))

            catf = spool.tile([128, 1], F32, tag=f"catf{h}", bufs=1)
            nc.vector.tensor_copy(catf[:], cat_t[:, h:h + 1])
            catf_h.append(catf)

        # --- phase 4: center chunks close the accumulation; bias+relu fused ---
        for c in range(HC):
            for mt in range(2):
                nc.tensor.matmul(
                    accs[mt][:],
                    lhsT=wbT_t[:, c * INNER + mt * 128: c * INNER + (mt + 1) * 128],
                    rhs=featT[:, c * 256:(c + 1) * 256],
                    start=False, stop=(c == HC - 1),
                )
        for mt in range(2):
            nc.scalar.activation(hiddenT[:, mt * 256:(mt + 1) * 256], accs[mt][:],
                                 mybir.ActivationFunctionType.Relu,
                                 bias=bb_t[:, mt:mt + 1], scale=1.0)

        # --- phase 5: expert heads + per-sample selection ---
        out3 = pool.tile([128, 2 * NB_LABELS], F32)  # [p, h*3 + n]
        for h in range(2):
            b0 = h * 128
            mask36 = spool.tile([128, NE], F32, tag="mask36")
            nc.vector.tensor_scalar(mask36[:], io36f, catf_h[h][:, :1], None,
                                    op0=mybir.AluOpType.is_equal)
            ps36 = ps36p.tile([128, NE], F32, tag="ps36")
            nc.tensor.matmul(ps36[:], lhsT=hiddenT[:, b0:b0 + 128],
                             rhs=wexpA[:], start=True, stop=False)
            nc.tensor.matmul(ps36[:], lhsT=hiddenT[:, 256 + b0:256 + b0 + 128],
                             rhs=wexpB[:], start=False, stop=False)
            nc.tensor.matmul(ps36[:], lhsT=ones1[:, b0:b0 + 128],
                             rhs=wexpC[:], start=False, stop=True)

            prod = spool.tile([128, NE], F32, tag="prod")
            nc.vector.tensor_tensor(out=prod[:], in0=ps36[:], in1=mask36[:],
                                    op=mybir.AluOpType.mult)
            nc.vector.tensor_reduce(
                out=out3[:, h * NB_LABELS:(h + 1) * NB_LABELS],
                in_=prod[:].rearrange("p (e n) -> p n e", n=NB_LABELS),
                axis=mybir.AxisListType.X, op=mybir.AluOpType.add)
        nc.sync.dma_start(out.rearrange("(h p) n -> p h n", p=128),
                          out3[:].rearrange("p (h n) -> p h n", n=NB_LABELS))

    nc.compile()
    return nc


_NC = None


def _get_nc():
    global _NC
    if _NC is None:
        _NC = _build()
    return _NC


def _const_blobs():
    cstf = np.zeros((128, C_NF), dtype=np.float32)
    cstf[:, C_ID:C_ID + 128] = np.eye(128, dtype=np.float32)
    cstf[:, C_IO8:C_IO8 + SPAN] = np.arange(SPAN, dtype=np.float32)[None, :]
    cstf[:, C_IO36:C_IO36 + NE] = np.repeat(
        np.arange(NB_EXPERTS, dtype=np.float32), NB_LABELS)[None, :]
    return cstf


def _prep_inputs(embeddings, position_indexes, categories, W_base, b_base,
                 W_experts, b_experts):
    emb = np.ascontiguousarray(np.asarray(embeddings, dtype=np.float32)).reshape(
        NCORES, BC * S, H)
    pos = np.ascontiguousarray(np.asarray(position_indexes).astype(np.int32)).reshape(
        NCORES, BC, 2)
    cat = np.ascontiguousarray(np.asarray(categories).astype(np.int32)).reshape(
        NCORES, BC, 1)
    # wbT_host[p, c*INNER+m] = W_base[m, c*128+p]; shipped as [3H, INNER] rows
    # grouped so the device DMA is a single contiguous [128, 18*256] copy.
    wb = np.asarray(W_base, dtype=np.float32)  # [INNER, 3H]
    wbT = np.ascontiguousarray(
        wb.T.reshape(KC, 128, INNER).transpose(1, 0, 2).reshape(128, KC * INNER)
    ).reshape(F3H, INNER)  # same bytes, declared [3H, INNER] for the DRAM tensor
    bb = np.ascontiguousarray(np.asarray(b_base, dtype=np.float32))
    we = np.asarray(W_experts, dtype=np.float32)  # [12, 3, INNER]
    be = np.asarray(b_experts, dtype=np.float32)  # [12, 3]
    wexpT = np.concatenate(
        [we.transpose(2, 0, 1).reshape(INNER, NE), be.reshape(1, NE)], axis=0)
    wexpT = np.ascontiguousarray(wexpT)  # [INNER+1, 36]
    cstf = _const_blobs()

    # Per-core gather row indices [128, 6]: col h*3+ci holds the first row of
    # span chunk ci ([0:4), [4:6), [6:8)) for sample h*128+p, or BIG when the
    # span doesn't reach that chunk (dropped by the DGE bounds check).
    BIG = 100000
    starts = pos[:, :, 0].astype(np.int64)                  # [NCORES, BC]
    lens = (pos[:, :, 1] - pos[:, :, 0]).astype(np.int64)
    base = np.arange(BC, dtype=np.int64) * S
    i0 = base[None, :] + starts
    c1 = np.where(lens > 4, i0 + 4, BIG)
    c2 = np.where(lens > 6, i0 + 6, BIG)
    gidx = np.stack([i0, c1, c2], axis=-1).reshape(NCORES, 2, 128, 3)
    gidx = np.ascontiguousarray(
        gidx.transpose(0, 2, 1, 3).reshape(NCORES, 128, 6).astype(np.int32))

    return [
        {"emb": emb[i], "pos": pos[i], "cat": cat[i], "wbT": wbT, "bb": bb,
         "wexpT": wexpT, "cstf": cstf, "gidx": gidx[i],
         "onesd": np.ones((1, 256), dtype=np.float32)}
        for i in range(NCORES)
    ]


def _run(in_maps, **kw):
    nc = _get_nc()
    return run_bass_kernel_spmd(nc, in_maps, core_ids=list(range(NCORES)), **kw)


def kernel(embeddings, position_indexes, categories, W_base, b_base, W_experts,
           b_experts):
    in_maps = _prep_inputs(embeddings, position_indexes, categories, W_base,
                           b_base, W_experts, b_experts)
    res = _run(in_maps)
    return np.concatenate([r["out"] for r in res.results], axis=0)


# revision 55
# speedup vs baseline: 1.1562x; 1.1562x over previous
"""Trainium2 Bass kernel for nn_BertClassifier_77309411685.

Data-parallel over 8 NeuronCores: each core handles 256 samples of the
2048-sample batch; the small base linear and 12 expert heads are replicated.

Per-core device algorithm (B_c=256 samples, processed as 2 halves of 128):
  1. indirect-DMA gather: for each sample, the 8 consecutive embedding rows
     starting at `start` (spans are 1..8 long and start <= S-9, so 8 rows are
     always in bounds), split into two 4-row chunks so the masked-mean can
     start while the second chunk is still in flight. One sample per
     partition.
  2. masked mean over the span via fused DVE multiply-accumulate with
     per-partition weights (i < len) / len.
  3. two static context rows loaded with strided DMA.
  4. PE transposes build featT [3H=2304, 256] (feature-major) from the
     sample-major center/context tiles; 3 transposes share one PSUM bank and
     drain with a single strided copy.
  5. base linear: hiddenT[inner, b] = relu(W_base @ feat + b_base) via 18
     K-chunk matmuls per 128-wide inner tile, bias+relu fused in the
     PSUM->SBUF activation.
  6. expert heads: compute all 12 experts at once, out36[b, e*3+n], with the
     bias folded in as an extra K=1 matmul against a ones row; then select
     the right expert per sample with an is_equal mask and a strided reduce.

Constants (identity matrix, iota ramps, per-partition row bases) are shipped
as one small DRAM input instead of being built with gpsimd ops on device.
"""

import numpy as np
from contextlib import ExitStack

import concourse.bass as bass
import concourse.tile as tile
from concourse import bacc, mybir
from concourse.bass import IndirectOffsetOnAxis
from concourse.bass_utils import run_bass_kernel_spmd

F32 = mybir.dt.float32
I32 = mybir.dt.int32

B, S, H = 2048, 256, 768
INNER, NB_CTX, NB_EXPERTS, NB_LABELS = 256, 2, 12, 3
NCORES = 8
BC = B // NCORES           # 256 samples per core
F3H = (NB_CTX + 1) * H     # 2304
KC = F3H // 128            # 18 contraction chunks
NE = NB_EXPERTS * NB_LABELS  # 36
SPAN = 8                   # max span length; always safe to gather 8 rows
HC = H // 128              # 6 h-chunks per feature block

# The reference picks 2 static context positions host-side with this exact rng.
CTX_IDX = [int(v) for v in np.random.default_rng(seed=0).choice(np.arange(S), size=NB_CTX)]

# Dtype used for the PE matmul operands (featT, weights, hiddenT).
#  float32: 4 cycles/row (2 half-speed passes) — exact baseline.
#  float32r: 1 cycle/row at N>=256 — reduced-mantissa single pass.
#  bfloat16: 1 cycle/row + fast weight load, half the SBUF traffic.
# Accumulation is always fp32 in PSUM; center/ctx stay fp32 until the single
# rounding point at the PSUM->featT copy.
MM_DT = mybir.dt.float32r
# Dtype the span gather lands in SBUF as (the DMA casts in flight when bf16 —
# halves the SBUF-port bytes of the SWDGE stream; measured no win, keep f32).
GATHER_DT = mybir.dt.float32

# Const blob layout (f32 columns): identity [0:128), io8f [128:136),
# io36f [136:172).  Separate int32 blob: rowbase [p, 0] = p*S.
C_ID, C_IO8, C_IO36, C_NF = 0, 128, 136, 172


def _build():
    nc = bacc.Bacc(
        "TRN2",
        target_bir_lowering=False,
        debug=False,
        enable_asserts=False,
        num_devices=NCORES,
    )
    emb = nc.dram_tensor("emb", [BC * S, H], F32, kind="ExternalInput").ap()
    pos = nc.dram_tensor("pos", [BC, 2], I32, kind="ExternalInput").ap()
    cat = nc.dram_tensor("cat", [BC, 1], I32, kind="ExternalInput").ap()
    wbT = nc.dram_tensor("wbT", [F3H, INNER], F32, kind="ExternalInput").ap()
    bb = nc.dram_tensor("bb", [INNER], F32, kind="ExternalInput").ap()
    wexpT = nc.dram_tensor("wexpT", [INNER + 1, NE], F32, kind="ExternalInput").ap()
    cstf = nc.dram_tensor("cstf", [128, C_NF], F32, kind="ExternalInput").ap()
    onesd = nc.dram_tensor("onesd", [1, 256], F32, kind="ExternalInput").ap()
    # Host-computed gather row indices, col h*3+ci: chunk start rows, with the
    # skip marker (BIG, beyond the bounds check) baked in for chunks 1/2 when
    # the span doesn't reach them.
    gidx = nc.dram_tensor("gidx", [128, 6], I32, kind="ExternalInput").ap()
    out = nc.dram_tensor("out", [BC, NB_LABELS], F32, kind="ExternalOutput").ap()

    emb3d = emb.rearrange("(b s) h -> b s h", s=S)
    TD = MM_DT
    GDT = GATHER_DT

    def asTD(dram_ap):
        # f32 DRAM bits reinterpreted as the PE dtype (f32r shares the layout).
        return dram_ap.bitcast(TD) if TD == mybir.dt.float32r else dram_ap

    with tile.TileContext(nc) as tc, ExitStack() as ctx:
        pool = ctx.enter_context(tc.tile_pool(name="main", bufs=1))
        gpool = ctx.enter_context(tc.tile_pool(name="gp", bufs=2))
        spool = ctx.enter_context(tc.tile_pool(name="small", bufs=2))
        pst = ctx.enter_context(tc.tile_pool(name="pst", bufs=2, space="PSUM"))
        psh = ctx.enter_context(tc.tile_pool(name="psh", bufs=2, space="PSUM"))
        ps36p = ctx.enter_context(tc.tile_pool(name="ps36p", bufs=2, space="PSUM"))

        # --- phase 0: tiny front-of-queue loads the gather depends on ---
        gidx_t = pool.tile([128, 6], I32)
        nc.sync.dma_start(gidx_t[:], gidx[:, :])
        pos_t = pool.tile([128, 4], I32)  # [p, h*2 + j] = pos[h*128+p, j]
        nc.sync.dma_start(pos_t[:].rearrange("p (h j) -> p h j", j=2),
                          pos.rearrange("(h p) j -> p h j", p=128))
        cstf_t = pool.tile([128, C_NF], F32)
        nc.sync.dma_start(cstf_t[:], cstf[:, :])
        cat_t = pool.tile([128, 2], I32)  # [p, h] = cat[h*128+p]
        nc.sync.dma_start(cat_t[:].rearrange("p (h j) -> p h j", j=1),
                          cat.rearrange("(h p) j -> p h j", p=128))

        io8f = cstf_t[:, C_IO8:C_IO8 + SPAN]
        io36f = cstf_t[:, C_IO36:C_IO36 + NE]
        # Identity for PE transposes, in the PE dtype (separate tile so the
        # fp32r verifier sees a rounded producer).
        id_t = pool.tile([128, 128], TD)
        nc.sync.dma_start(id_t[:], asTD(cstf[:, C_ID:C_ID + 128]))
        identity = id_t[:]

        # Pre-zero the conditional gather chunks first thing on DVE: skipped
        # samples keep zeros, and the zero span weights keep them out of the
        # mean. Must land before the conditional gathers start writing.
        gz = []
        for ci in (1, 2):
            for h in range(2):
                g = gpool.tile([128, 2 * H], GDT, tag=f"g{h}{ci}", bufs=1)
                nc.vector.memset(g[:], 0.0)
                gz.append(g)

        # --- phase 1: per-half index chains + gathers, earliest possible ---
        # Row chunks per sample: [0:4), [4:6), [6:8).  Chunks 1/2 are skipped
        # per-sample via the DGE bounds check when the span doesn't reach them
        # (len<=4 / len<=6); their tiles are pre-zeroed so skipped lanes stay 0
        # and the zero weights keep them out of the mean.
        g_chunks = [[None, None, None], [None, None, None]]
        # chunk 0 (always needed) goes out as soon as its indices land.
        # The two halves ride different SWDGE queue rows so each SDMA engine
        # round-robins between two descriptor streams (hides HBM read latency).
        for h in range(2):
            g0 = gpool.tile([128, 4 * H], GDT, tag=f"g{h}0", bufs=1)
            nc.gpsimd.indirect_dma_start(
                out=g0[:], out_offset=None, in_=emb,
                in_offset=IndirectOffsetOnAxis(ap=gidx_t[:, 3 * h:3 * h + 1], axis=0),
            )
            g_chunks[h][0] = g0

        # conditional chunks: gather with the bounds check dropping the
        # per-sample skip-marked indices; interleave halves so each half's
        # last chunk lands as early as possible
        for ci in (1, 2):
            for h in range(2):
                g = gz[(ci - 1) * 2 + h]
                nc.gpsimd.indirect_dma_start(
                    out=g[:], out_offset=None, in_=emb,
                    in_offset=IndirectOffsetOnAxis(
                        ap=gidx_t[:, 3 * h + ci:3 * h + ci + 1], axis=0),
                    bounds_check=BC * S - 1, oob_is_err=False,
                )
                g_chunks[h][ci] = g

        w8_h = []
        for h in range(2):
            # span weights w8[p, i] = (i < len) / len
            len_i = spool.tile([128, 1], I32, tag=f"leni{h}", bufs=1)
            nc.vector.tensor_tensor(out=len_i[:], in0=pos_t[:, 2 * h + 1:2 * h + 2],
                                    in1=pos_t[:, 2 * h:2 * h + 1],
                                    op=mybir.AluOpType.subtract)
            len_f = spool.tile([128, 1], F32, tag=f"lenf{h}", bufs=1)
            nc.vector.tensor_copy(len_f[:], len_i[:])
            rcp = spool.tile([128, 1], F32, tag=f"rcp{h}", bufs=1)
            nc.vector.reciprocal(rcp[:], len_f[:])
            w8 = spool.tile([128, SPAN], F32, tag=f"w8{h}", bufs=1)
            nc.vector.tensor_scalar(w8[:], io8f, len_f[:, :1], rcp[:, :1],
                                    op0=mybir.AluOpType.is_lt,
                                    op1=mybir.AluOpType.mult)
            w8_h.append(w8)

        # --- phase 2: context rows + replicated weights (overlap gathers) ---
        ctxs = []
        for h in range(2):
            b0 = h * 128
            ctx0 = gpool.tile([128, H], TD, tag=f"ctx0{h}", bufs=1)
            nc.sync.dma_start(ctx0[:], asTD(emb3d[b0:b0 + 128, CTX_IDX[0], :]))
            ctx1 = gpool.tile([128, H], TD, tag=f"ctx1{h}", bufs=1)
            nc.sync.dma_start(ctx1[:], asTD(emb3d[b0:b0 + 128, CTX_IDX[1], :]))
            ctxs.append((ctx0, ctx1))

        # wbT is shipped pre-laid-out: wbT_host[p, c*INNER+m] = W_base[m, c*128+p].
        # Split into 6 medium DMAs so the packets interleave gently with the
        # concurrent indirect gathers.
        wbT_t = pool.tile([128, KC * INNER], TD)
        wbT_c = wbT.rearrange("(p x) m -> p (x m)", p=128)
        step = KC * INNER // 6
        for j in range(6):
            sl = slice(j * step, (j + 1) * step)
            if TD == mybir.dt.bfloat16:
                nc.gpsimd.dma_start(wbT_t[:, sl], wbT_c[:, sl])
            else:
                nc.sync.dma_start(wbT_t[:, sl], asTD(wbT_c[:, sl]))
        bb_t = pool.tile([128, 2], F32)  # bb_t[p, t] = b_base[t*128 + p]
        nc.sync.dma_start(bb_t[:], bb.rearrange("(t p) -> p t", p=128))
        wexpA = pool.tile([128, NE], TD)
        wexpB = pool.tile([128, NE], TD)
        wexpC = pool.tile([1, NE], TD)
        if TD == mybir.dt.bfloat16:
            nc.gpsimd.dma_start(wexpA[:], wexpT[0:128, :])
            nc.gpsimd.dma_start(wexpB[:], wexpT[128:256, :])
            nc.gpsimd.dma_start(wexpC[:], wexpT[256:257, :])
        else:
            nc.sync.dma_start(wexpA[:], asTD(wexpT[0:128, :]))
            nc.sync.dma_start(wexpB[:], asTD(wexpT[128:256, :]))
            nc.sync.dma_start(wexpC[:], asTD(wexpT[256:257, :]))
        ones1 = pool.tile([1, 256], TD)
        if TD == mybir.dt.float32r:
            nc.sync.dma_start(ones1[:], asTD(onesd[:, :]))
        else:
            nc.vector.memset(ones1[:], 1.0)

        # --- phase 3a: ctx transposes + copies (their data lands early) ---
        featT = pool.tile([128, KC * 256], TD)
        featT3 = featT[:].rearrange("p (si rest) -> p si rest", si=3)
        for h in range(2):
            ctx0, ctx1 = ctxs[h]
            for c in range(HC):
                tpc = pst.tile([128, 2 * 128], TD, tag="tpc")
                for si, src in enumerate((ctx0, ctx1)):
                    nc.tensor.transpose(tpc[:, si * 128:(si + 1) * 128],
                                        src[:, c * 128:(c + 1) * 128], identity)
                col = c * 256 + h * 128
                nc.scalar.copy(featT3[:, 1:3, col:col + 128],
                               tpc[:].rearrange("p (si x) -> p si x", si=2))

        # --- phase 3b: ctx part of the base linear runs during the gather ---
        hiddenT = pool.tile([128, 2 * 256], TD)
        accs = [psh.tile([128, 256], F32, tag=f"acc{mt}", bufs=1, name=f"acc{mt}")
                for mt in range(2)]
        for c in range(HC, KC):
            for mt in range(2):
                nc.tensor.matmul(
                    accs[mt][:],
                    lhsT=wbT_t[:, c * INNER + mt * 128: c * INNER + (mt + 1) * 128],
                    rhs=featT[:, c * 256:(c + 1) * 256],
                    start=(c == HC), stop=False,
                )

        # --- phase 3c: masked mean + center transposes ---
        catf_h = []
        for h in range(2):
            w8 = w8_h[h]
            # accA: rows 0-3 (chunk 0 lands first)
            accA = gpool.tile([128, H], F32, tag=f"accA{h}", bufs=1)
            nc.vector.tensor_scalar(accA[:], g_chunks[h][0][:, 0:H], w8[:, 0:1],
                                    None, op0=mybir.AluOpType.mult)
            for i in range(1, 4):
                off = i * H
                nc.vector.scalar_tensor_tensor(
                    out=accA[:], in0=g_chunks[h][0][:, off:off + H],
                    scalar=w8[:, i:i + 1], in1=accA[:],
                    op0=mybir.AluOpType.mult, op1=mybir.AluOpType.add)
            # accB: rows 4-7 (conditional chunks land last)
            accB = gpool.tile([128, H], F32, tag=f"accB{h}", bufs=1)
            nc.vector.tensor_scalar(accB[:], g_chunks[h][1][:, 0:H], w8[:, 4:5],
                                    None, op0=mybir.AluOpType.mult)
            for i, (ci, off) in enumerate([(1, H), (2, 0), (2, H)], start=5):
                nc.vector.scalar_tensor_tensor(
                    out=accB[:], in0=g_chunks[h][ci][:, off:off + H],
                    scalar=w8[:, i:i + 1], in1=accB[:],
                    op0=mybir.AluOpType.mult, op1=mybir.AluOpType.add)
            center = gpool.tile([128, H], TD, tag=f"center{h}", bufs=1)
            nc.vector.tensor_tensor(out=center[:], in0=accA[:], in1=accB[:],
                                    op=mybir.AluOpType.add)

            # center transposes; one ACT copy per h-chunk keeps DVE free
            for c in range(HC):
                tp = pst.tile([128, 128], TD, tag="tp")
                nc.tensor.transpose(tp[:], center[:, c * 128:(c + 1) * 128],
                                    identity)
                col = c * 256 + h * 128
                nc.scalar.copy(featT3[:, 0:1, col:col + 128],
                               tp[:].rearrange("p (si x) -> p si x", si=1))

            catf = spool.tile([128, 1], F32, tag=f"catf{h}", bufs=1)
            nc.vector.tensor_copy(catf[:], cat_t[:, h:h + 1])
            catf_h.append(catf)

        # --- phase 4: center chunks close the accumulation; bias+relu fused ---
        for c in range(HC):
            for mt in range(2):
                nc.tensor.matmul(
                    accs[mt][:],
                    lhsT=wbT_t[:, c * INNER + mt * 128: c * INNER + (mt + 1) * 128],
                    rhs=featT[:, c * 256:(c + 1) * 256],
                    start=False, stop=(c == HC - 1),
                )
        for mt in range(2):
            nc.scalar.activation(hiddenT[:, mt * 256:(mt + 1) * 256], accs[mt][:],
                                 mybir.ActivationFunctionType.Relu,
                                 bias=bb_t[:, mt:mt + 1], scale=1.0)

        # --- phase 5: expert heads + per-sample selection ---
        out3 = pool.tile([128, 2 * NB_LABELS], F32)  # [p, h*3 + n]
        for h in range(2):
            b0 = h * 128
            mask36 = spool.tile([128, NE], F32, tag="mask36")
            nc.vector.tensor_scalar(mask36[:], io36f, catf_h[h][:, :1], None,
                                    op0=mybir.AluOpType.is_equal)
            ps36 = ps36p.tile([128, NE], F32, tag="ps36")
            nc.tensor.matmul(ps36[:], lhsT=hiddenT[:, b0:b0 + 128],
                             rhs=wexpA[:], start=True, stop=False)
            nc.tensor.matmul(ps36[:], lhsT=hiddenT[:, 256 + b0:256 + b0 + 128],
                             rhs=wexpB[:], start=False, stop=False)
            nc.tensor.matmul(ps36[:], lhsT=ones1[:, b0:b0 + 128],
                             rhs=wexpC[:], start=False, stop=True)

            prod = spool.tile([128, NE], F32, tag="prod")
            nc.vector.tensor_tensor(out=prod[:], in0=ps36[:], in1=mask36[:],
                                    op=mybir.AluOpType.mult)
            nc.vector.tensor_reduce(
                out=out3[:, h * NB_LABELS:(h + 1) * NB_LABELS],
                in_=prod[:].rearrange("p (e n) -> p n e", n=NB_LABELS),
                axis=mybir.AxisListType.X, op=mybir.AluOpType.add)
        nc.sync.dma_start(out.rearrange("(h p) n -> p h n", p=128),
                          out3[:].rearrange("p (h n) -> p h n", n=NB_LABELS))

    nc.compile()
    return nc


_NC = None


def _get_nc():
    global _NC
    if _NC is None:
        _NC = _build()
    return _NC


def _const_blobs():
    cstf = np.zeros((128, C_NF), dtype=np.float32)
    cstf[:, C_ID:C_ID + 128] = np.eye(128, dtype=np.float32)
    cstf[:, C_IO8:C_IO8 + SPAN] = np.arange(SPAN, dtype=np.float32)[None, :]
    cstf[:, C_IO36:C_IO36 + NE] = np.repeat(
        np.arange(NB_EXPERTS, dtype=np.float32), NB_LABELS)[None, :]
    return cstf


def _prep_inputs(embeddings, position_indexes, categories, W_base, b_base,
                 W_experts, b_experts):
    emb = np.ascontiguousarray(np.asarray(embeddings, dtype=np.float32)).reshape(
        NCORES, BC * S, H)
    pos = np.ascontiguousarray(np.asarray(position_indexes).astype(np.int32)).reshape(
        NCORES, BC, 2)
    cat = np.ascontiguousarray(np.asarray(categories).astype(np.int32)).reshape(
        NCORES, BC, 1)
    # wbT_host[p, c*INNER+m] = W_base[m, c*128+p]; shipped as [3H, INNER] rows
    # grouped so the device DMA is a single contiguous [128, 18*256] copy.
    wb = np.asarray(W_base, dtype=np.float32)  # [INNER, 3H]
    wbT = np.ascontiguousarray(
        wb.T.reshape(KC, 128, INNER).transpose(1, 0, 2).reshape(128, KC * INNER)
    ).reshape(F3H, INNER)  # same bytes, declared [3H, INNER] for the DRAM tensor
    bb = np.ascontiguousarray(np.asarray(b_base, dtype=np.float32))
    we = np.asarray(W_experts, dtype=np.float32)  # [12, 3, INNER]
    be = np.asarray(b_experts, dtype=np.float32)  # [12, 3]
    wexpT = np.concatenate(
        [we.transpose(2, 0, 1).reshape(INNER, NE), be.reshape(1, NE)], axis=0)
    wexpT = np.ascontiguousarray(wexpT)  # [INNER+1, 36]
    cstf = _const_blobs()

    # Per-core gather row indices [128, 6]: col h*3+ci holds the first row of
    # span chunk ci ([0:4), [4:6), [6:8)) for sample h*128+p, or BIG when the
    # span doesn't reach that chunk (dropped by the DGE bounds check).
    BIG = 100000
    starts = pos[:, :, 0].astype(np.int64)                  # [NCORES, BC]
    lens = (pos[:, :, 1] - pos[:, :, 0]).astype(np.int64)
    base = np.arange(BC, dtype=np.int64) * S
    i0 = base[None, :] + starts
    c1 = np.where(lens > 4, i0 + 4, BIG)
    c2 = np.where(lens > 6, i0 + 6, BIG)
    gidx = np.stack([i0, c1, c2], axis=-1).reshape(NCORES, 2, 128, 3)
    gidx = np.ascontiguousarray(
        gidx.transpose(0, 2, 1, 3).reshape(NCORES, 128, 6).astype(np.int32))

    return [
        {"emb": emb[i], "pos": pos[i], "cat": cat[i], "wbT": wbT, "bb": bb,
         "wexpT": wexpT, "cstf": cstf, "gidx": gidx[i],
         "onesd": np.ones((1, 256), dtype=np.float32)}
        for i in range(NCORES)
    ]


def _run(in_maps, **kw):
    nc = _get_nc()
    return run_bass_kernel_spmd(nc, in_maps, core_ids=list(range(NCORES)), **kw)


def kernel(embeddings, position_indexes, categories, W_base, b_base, W_experts,
           b_experts):
    in_maps = _prep_inputs(embeddings, position_indexes, categories, W_base,
                           b_base, W_experts, b_experts)
    res = _run(in_maps)
    return np.concatenate([r["out"] for r in res.results], axis=0)


# revision 56
# speedup vs baseline: 1.1833x; 1.0235x over previous
"""Trainium2 Bass kernel for nn_BertClassifier_77309411685.

Data-parallel over 8 NeuronCores: each core handles 256 samples of the
2048-sample batch; the small base linear and 12 expert heads are replicated.

Per-core device algorithm (B_c=256 samples, processed as 2 halves of 128):
  1. indirect-DMA gather: for each sample, the 8 consecutive embedding rows
     starting at `start` (spans are 1..8 long and start <= S-9, so 8 rows are
     always in bounds), split into two 4-row chunks so the masked-mean can
     start while the second chunk is still in flight. One sample per
     partition.
  2. masked mean over the span via fused DVE multiply-accumulate with
     per-partition weights (i < len) / len.
  3. two static context rows loaded with strided DMA.
  4. PE transposes build featT [3H=2304, 256] (feature-major) from the
     sample-major center/context tiles; 3 transposes share one PSUM bank and
     drain with a single strided copy.
  5. base linear: hiddenT[inner, b] = relu(W_base @ feat + b_base) via 18
     K-chunk matmuls per 128-wide inner tile, bias+relu fused in the
     PSUM->SBUF activation.
  6. expert heads: compute all 12 experts at once, out36[b, e*3+n], with the
     bias folded in as an extra K=1 matmul against a ones row; then select
     the right expert per sample with an is_equal mask and a strided reduce.

Constants (identity matrix, iota ramps, per-partition row bases) are shipped
as one small DRAM input instead of being built with gpsimd ops on device.
"""

import numpy as np
from contextlib import ExitStack

import concourse.bass as bass
import concourse.tile as tile
from concourse import bacc, mybir
from concourse.bass import IndirectOffsetOnAxis
from concourse.bass_utils import run_bass_kernel_spmd

F32 = mybir.dt.float32
I32 = mybir.dt.int32

B, S, H = 2048, 256, 768
INNER, NB_CTX, NB_EXPERTS, NB_LABELS = 256, 2, 12, 3
NCORES = 8
BC = B // NCORES           # 256 samples per core
F3H = (NB_CTX + 1) * H     # 2304
KC = F3H // 128            # 18 contraction chunks
NE = NB_EXPERTS * NB_LABELS  # 36
SPAN = 8                   # max span length; always safe to gather 8 rows
HC = H // 128              # 6 h-chunks per feature block

# The reference picks 2 static context positions host-side with this exact rng.
CTX_IDX = [int(v) for v in np.random.default_rng(seed=0).choice(np.arange(S), size=NB_CTX)]

# Dtype used for the PE matmul operands (featT, weights, hiddenT).
#  float32: 4 cycles/row (2 half-speed passes) — exact baseline.
#  float32r: 1 cycle/row at N>=256 — reduced-mantissa single pass.
#  bfloat16: 1 cycle/row + fast weight load, half the SBUF traffic.
# Accumulation is always fp32 in PSUM; center/ctx stay fp32 until the single
# rounding point at the PSUM->featT copy.
MM_DT = mybir.dt.float32r
# Dtype the span gather lands in SBUF as (the DMA casts in flight when bf16 —
# halves the SBUF-port bytes of the SWDGE stream; measured no win, keep f32).
GATHER_DT = mybir.dt.float32

# Const blob layout (f32 columns): identity [0:128), io8f [128:136),
# io36f [136:172).  Separate int32 blob: rowbase [p, 0] = p*S.
C_ID, C_IO8, C_IO36, C_NF = 0, 128, 136, 172


def _build():
    nc = bacc.Bacc(
        "TRN2",
        target_bir_lowering=False,
        debug=False,
        enable_asserts=False,
        num_devices=NCORES,
    )
    emb = nc.dram_tensor("emb", [BC * S, H], F32, kind="ExternalInput").ap()
    pos = nc.dram_tensor("pos", [BC, 2], I32, kind="ExternalInput").ap()
    cat = nc.dram_tensor("cat", [BC, 1], I32, kind="ExternalInput").ap()
    wbT = nc.dram_tensor("wbT", [F3H, INNER], F32, kind="ExternalInput").ap()
    bb = nc.dram_tensor("bb", [INNER], F32, kind="ExternalInput").ap()
    wexpT = nc.dram_tensor("wexpT", [INNER + 1, NE], F32, kind="ExternalInput").ap()
    cstf = nc.dram_tensor("cstf", [128, C_NF], F32, kind="ExternalInput").ap()
    onesd = nc.dram_tensor("onesd", [1, 256], F32, kind="ExternalInput").ap()
    # Host-computed gather row indices, col h*3+ci: chunk start rows, with the
    # skip marker (BIG, beyond the bounds check) baked in for chunks 1/2 when
    # the span doesn't reach them.
    gidx = nc.dram_tensor("gidx", [128, 6], I32, kind="ExternalInput").ap()
    out = nc.dram_tensor("out", [BC, NB_LABELS], F32, kind="ExternalOutput").ap()

    emb3d = emb.rearrange("(b s) h -> b s h", s=S)
    TD = MM_DT
    GDT = GATHER_DT

    def asTD(dram_ap):
        # f32 DRAM bits reinterpreted as the PE dtype (f32r shares the layout).
        return dram_ap.bitcast(TD) if TD == mybir.dt.float32r else dram_ap

    with tile.TileContext(nc) as tc, ExitStack() as ctx:
        pool = ctx.enter_context(tc.tile_pool(name="main", bufs=1))
        gpool = ctx.enter_context(tc.tile_pool(name="gp", bufs=2))
        spool = ctx.enter_context(tc.tile_pool(name="small", bufs=2))
        pst = ctx.enter_context(tc.tile_pool(name="pst", bufs=2, space="PSUM"))
        psh = ctx.enter_context(tc.tile_pool(name="psh", bufs=2, space="PSUM"))
        ps36p = ctx.enter_context(tc.tile_pool(name="ps36p", bufs=2, space="PSUM"))

        # --- phase 0: tiny front-of-queue loads the gather depends on ---
        gidx_t = pool.tile([128, 6], I32)
        nc.sync.dma_start(gidx_t[:], gidx[:, :])
        pos_t = pool.tile([128, 4], I32)  # [p, h*2 + j] = pos[h*128+p, j]
        nc.sync.dma_start(pos_t[:].rearrange("p (h j) -> p h j", j=2),
                          pos.rearrange("(h p) j -> p h j", p=128))
        cstf_t = pool.tile([128, C_NF], F32)
        nc.sync.dma_start(cstf_t[:], cstf[:, :])
        cat_t = pool.tile([128, 2], I32)  # [p, h] = cat[h*128+p]
        nc.sync.dma_start(cat_t[:].rearrange("p (h j) -> p h j", j=1),
                          cat.rearrange("(h p) j -> p h j", p=128))

        io8f = cstf_t[:, C_IO8:C_IO8 + SPAN]
        io36f = cstf_t[:, C_IO36:C_IO36 + NE]
        # Identity for PE transposes, in the PE dtype (separate tile so the
        # fp32r verifier sees a rounded producer).
        id_t = pool.tile([128, 128], TD)
        nc.sync.dma_start(id_t[:], asTD(cstf[:, C_ID:C_ID + 128]))
        identity = id_t[:]

        # Pre-zero the conditional gather chunks first thing on DVE: skipped
        # samples keep zeros, and the zero span weights keep them out of the
        # mean. Must land before the conditional gathers start writing.
        gz = []
        for ci in (1, 2):
            for h in range(2):
                g = gpool.tile([128, 2 * H], GDT, tag=f"g{h}{ci}", bufs=1)
                nc.vector.memset(g[:], 0.0)
                gz.append(g)

        # --- phase 1: per-half index chains + gathers, earliest possible ---
        # Row chunks per sample: [0:4), [4:6), [6:8).  Chunks 1/2 are skipped
        # per-sample via the DGE bounds check when the span doesn't reach them
        # (len<=4 / len<=6); their tiles are pre-zeroed so skipped lanes stay 0
        # and the zero weights keep them out of the mean.
        g_chunks = [[None, None, None], [None, None, None]]
        # chunk 0 (always needed) goes out as soon as its indices land.
        # The two halves ride different SWDGE queue rows so each SDMA engine
        # round-robins between two descriptor streams (hides HBM read latency).
        for h in range(2):
            g0 = gpool.tile([128, 4 * H], GDT, tag=f"g{h}0", bufs=1)
            nc.gpsimd.indirect_dma_start(
                out=g0[:], out_offset=None, in_=emb,
                in_offset=IndirectOffsetOnAxis(ap=gidx_t[:, 3 * h:3 * h + 1], axis=0),
            )
            g_chunks[h][0] = g0

        # conditional chunks: gather with the bounds check dropping the
        # per-sample skip-marked indices; interleave halves so each half's
        # last chunk lands as early as possible
        for ci in (1, 2):
            for h in range(2):
                g = gz[(ci - 1) * 2 + h]
                nc.gpsimd.indirect_dma_start(
                    out=g[:], out_offset=None, in_=emb,
                    in_offset=IndirectOffsetOnAxis(
                        ap=gidx_t[:, 3 * h + ci:3 * h + ci + 1], axis=0),
                    bounds_check=BC * S - 1, oob_is_err=False,
                )
                g_chunks[h][ci] = g

        w8_h = []
        for h in range(2):
            # span weights w8[p, i] = (i < len) / len
            len_i = spool.tile([128, 1], I32, tag=f"leni{h}", bufs=1)
            nc.vector.tensor_tensor(out=len_i[:], in0=pos_t[:, 2 * h + 1:2 * h + 2],
                                    in1=pos_t[:, 2 * h:2 * h + 1],
                                    op=mybir.AluOpType.subtract)
            len_f = spool.tile([128, 1], F32, tag=f"lenf{h}", bufs=1)
            nc.vector.tensor_copy(len_f[:], len_i[:])
            rcp = spool.tile([128, 1], F32, tag=f"rcp{h}", bufs=1)
            nc.vector.reciprocal(rcp[:], len_f[:])
            w8 = spool.tile([128, SPAN], F32, tag=f"w8{h}", bufs=1)
            nc.vector.tensor_scalar(w8[:], io8f, len_f[:, :1], rcp[:, :1],
                                    op0=mybir.AluOpType.is_lt,
                                    op1=mybir.AluOpType.mult)
            w8_h.append(w8)

        # --- phase 2: context rows + replicated weights (overlap gathers) ---
        ctxs = []
        for h in range(2):
            b0 = h * 128
            ctx0 = gpool.tile([128, H], TD, tag=f"ctx0{h}", bufs=1)
            nc.sync.dma_start(ctx0[:], asTD(emb3d[b0:b0 + 128, CTX_IDX[0], :]))
            ctx1 = gpool.tile([128, H], TD, tag=f"ctx1{h}", bufs=1)
            nc.sync.dma_start(ctx1[:], asTD(emb3d[b0:b0 + 128, CTX_IDX[1], :]))
            ctxs.append((ctx0, ctx1))

        # wbT is shipped pre-laid-out: wbT_host[p, c*INNER+m] = W_base[m, c*128+p].
        # Split into 6 medium DMAs so the packets interleave gently with the
        # concurrent indirect gathers.
        wbT_t = pool.tile([128, KC * INNER], TD)
        wbT_c = wbT.rearrange("(p x) m -> p (x m)", p=128)
        step = KC * INNER // 6
        # ctx-chunk weights (cols 6*INNER..) are needed mid-gather by phase 3b;
        # center-chunk weights (cols 0..6*INNER) aren't needed until phase 4.
        for j in (2, 3, 4, 5, 0, 1):
            sl = slice(j * step, (j + 1) * step)
            if TD == mybir.dt.bfloat16:
                nc.gpsimd.dma_start(wbT_t[:, sl], wbT_c[:, sl])
            else:
                nc.sync.dma_start(wbT_t[:, sl], asTD(wbT_c[:, sl]))
        bb_t = pool.tile([128, 2], F32)  # bb_t[p, t] = b_base[t*128 + p]
        nc.sync.dma_start(bb_t[:], bb.rearrange("(t p) -> p t", p=128))
        wexpA = pool.tile([128, NE], TD)
        wexpB = pool.tile([128, NE], TD)
        wexpC = pool.tile([1, NE], TD)
        if TD == mybir.dt.bfloat16:
            nc.gpsimd.dma_start(wexpA[:], wexpT[0:128, :])
            nc.gpsimd.dma_start(wexpB[:], wexpT[128:256, :])
            nc.gpsimd.dma_start(wexpC[:], wexpT[256:257, :])
        else:
            nc.sync.dma_start(wexpA[:], asTD(wexpT[0:128, :]))
            nc.sync.dma_start(wexpB[:], asTD(wexpT[128:256, :]))
            nc.sync.dma_start(wexpC[:], asTD(wexpT[256:257, :]))
        ones1 = pool.tile([1, 256], TD)
        if TD == mybir.dt.float32r:
            nc.sync.dma_start(ones1[:], asTD(onesd[:, :]))
        else:
            nc.vector.memset(ones1[:], 1.0)

        # --- phase 3a: ctx transposes + copies (their data lands early) ---
        featT = pool.tile([128, KC * 256], TD)
        featT3 = featT[:].rearrange("p (si rest) -> p si rest", si=3)
        for h in range(2):
            ctx0, ctx1 = ctxs[h]
            for c in range(HC):
                tpc = pst.tile([128, 2 * 128], TD, tag="tpc")
                for si, src in enumerate((ctx0, ctx1)):
                    nc.tensor.transpose(tpc[:, si * 128:(si + 1) * 128],
                                        src[:, c * 128:(c + 1) * 128], identity)
                col = c * 256 + h * 128
                nc.scalar.copy(featT3[:, 1:3, col:col + 128],
                               tpc[:].rearrange("p (si x) -> p si x", si=2))

        # --- phase 3b: ctx part of the base linear runs during the gather ---
        hiddenT = pool.tile([128, 2 * 256], TD)
        accs = [psh.tile([128, 256], F32, tag=f"acc{mt}", bufs=1, name=f"acc{mt}")
                for mt in range(2)]
        for c in range(HC, KC):
            for mt in range(2):
                nc.tensor.matmul(
                    accs[mt][:],
                    lhsT=wbT_t[:, c * INNER + mt * 128: c * INNER + (mt + 1) * 128],
                    rhs=featT[:, c * 256:(c + 1) * 256],
                    start=(c == HC), stop=False,
                )

        # --- phase 3c: masked mean + center transposes ---
        catf_h = []
        for h in range(2):
            w8 = w8_h[h]
            # accA: rows 0-3 (chunk 0 lands first)
            accA = gpool.tile([128, H], F32, tag=f"accA{h}", bufs=1)
            nc.vector.tensor_scalar(accA[:], g_chunks[h][0][:, 0:H], w8[:, 0:1],
                                    None, op0=mybir.AluOpType.mult)
            for i in range(1, 4):
                off = i * H
                nc.vector.scalar_tensor_tensor(
                    out=accA[:], in0=g_chunks[h][0][:, off:off + H],
                    scalar=w8[:, i:i + 1], in1=accA[:],
                    op0=mybir.AluOpType.mult, op1=mybir.AluOpType.add)
            # accB: rows 4-7 (conditional chunks land last)
            accB = gpool.tile([128, H], F32, tag=f"accB{h}", bufs=1)
            nc.vector.tensor_scalar(accB[:], g_chunks[h][1][:, 0:H], w8[:, 4:5],
                                    None, op0=mybir.AluOpType.mult)
            for i, (ci, off) in enumerate([(1, H), (2, 0), (2, H)], start=5):
                nc.vector.scalar_tensor_tensor(
                    out=accB[:], in0=g_chunks[h][ci][:, off:off + H],
                    scalar=w8[:, i:i + 1], in1=accB[:],
                    op0=mybir.AluOpType.mult, op1=mybir.AluOpType.add)
            center = gpool.tile([128, H], TD, tag=f"center{h}", bufs=1)
            nc.vector.tensor_tensor(out=center[:], in0=accA[:], in1=accB[:],
                                    op=mybir.AluOpType.add)

            # center transposes; one ACT copy per h-chunk keeps DVE free
            for c in range(HC):
                tp = pst.tile([128, 128], TD, tag="tp")
                nc.tensor.transpose(tp[:], center[:, c * 128:(c + 1) * 128],
                                    identity)
                col = c * 256 + h * 128
                nc.scalar.copy(featT3[:, 0:1, col:col + 128],
                               tp[:].rearrange("p (si x) -> p si x", si=1))

            catf = spool.tile([128, 1], F32, tag=f"catf{h}", bufs=1)
            nc.vector.tensor_copy(catf[:], cat_t[:, h:h + 1])
            catf_h.append(catf)

        # --- phase 4: center chunks close the accumulation; bias+relu fused ---
        for c in range(HC):
            for mt in range(2):
                nc.tensor.matmul(
                    accs[mt][:],
                    lhsT=wbT_t[:, c * INNER + mt * 128: c * INNER + (mt + 1) * 128],
                    rhs=featT[:, c * 256:(c + 1) * 256],
                    start=False, stop=(c == HC - 1),
                )
        for mt in range(2):
            nc.scalar.activation(hiddenT[:, mt * 256:(mt + 1) * 256], accs[mt][:],
                                 mybir.ActivationFunctionType.Relu,
                                 bias=bb_t[:, mt:mt + 1], scale=1.0)

        # --- phase 5: expert heads + per-sample selection ---
        out3 = pool.tile([128, 2 * NB_LABELS], F32)  # [p, h*3 + n]
        for h in range(2):
            b0 = h * 128
            mask36 = spool.tile([128, NE], F32, tag="mask36")
            nc.vector.tensor_scalar(mask36[:], io36f, catf_h[h][:, :1], None,
                                    op0=mybir.AluOpType.is_equal)
            ps36 = ps36p.tile([128, NE], F32, tag="ps36")
            nc.tensor.matmul(ps36[:], lhsT=hiddenT[:, b0:b0 + 128],
                             rhs=wexpA[:], start=True, stop=False)
            nc.tensor.matmul(ps36[:], lhsT=hiddenT[:, 256 + b0:256 + b0 + 128],
                             rhs=wexpB[:], start=False, stop=False)
            nc.tensor.matmul(ps36[:], lhsT=ones1[:, b0:b0 + 128],
                             rhs=wexpC[:], start=False, stop=True)

            prod = spool.tile([128, NE], F32, tag="prod")
            nc.vector.tensor_tensor(out=prod[:], in0=ps36[:], in1=mask36[:],
                                    op=mybir.AluOpType.mult)
            nc.vector.tensor_reduce(
                out=out3[:, h * NB_LABELS:(h + 1) * NB_LABELS],
                in_=prod[:].rearrange("p (e n) -> p n e", n=NB_LABELS),
                axis=mybir.AxisListType.X, op=mybir.AluOpType.add)
        nc.sync.dma_start(out.rearrange("(h p) n -> p h n", p=128),
                          out3[:].rearrange("p (h n) -> p h n", n=NB_LABELS))

    nc.compile()
    return nc


_NC = None


def _get_nc():
    global _NC
    if _NC is None:
        _NC = _build()
    return _NC


def _const_blobs():
    cstf = np.zeros((128, C_NF), dtype=np.float32)
    cstf[:, C_ID:C_ID + 128] = np.eye(128, dtype=np.float32)
    cstf[:, C_IO8:C_IO8 + SPAN] = np.arange(SPAN, dtype=np.float32)[None, :]
    cstf[:, C_IO36:C_IO36 + NE] = np.repeat(
        np.arange(NB_EXPERTS, dtype=np.float32), NB_LABELS)[None, :]
    return cstf


def _prep_inputs(embeddings, position_indexes, categories, W_base, b_base,
                 W_experts, b_experts):
    emb = np.ascontiguousarray(np.asarray(embeddings, dtype=np.float32)).reshape(
        NCORES, BC * S, H)
    pos = np.ascontiguousarray(np.asarray(position_indexes).astype(np.int32)).reshape(
        NCORES, BC, 2)
    cat = np.ascontiguousarray(np.asarray(categories).astype(np.int32)).reshape(
        NCORES, BC, 1)
    # wbT_host[p, c*INNER+m] = W_base[m, c*128+p]; shipped as [3H, INNER] rows
    # grouped so the device DMA is a single contiguous [128, 18*256] copy.
    wb = np.asarray(W_base, dtype=np.float32)  # [INNER, 3H]
    wbT = np.ascontiguousarray(
        wb.T.reshape(KC, 128, INNER).transpose(1, 0, 2).reshape(128, KC * INNER)
    ).reshape(F3H, INNER)  # same bytes, declared [3H, INNER] for the DRAM tensor
    bb = np.ascontiguousarray(np.asarray(b_base, dtype=np.float32))
    we = np.asarray(W_experts, dtype=np.float32)  # [12, 3, INNER]
    be = np.asarray(b_experts, dtype=np.float32)  # [12, 3]
    wexpT = np.concatenate(
        [we.transpose(2, 0, 1).reshape(INNER, NE), be.reshape(1, NE)], axis=0)
    wexpT = np.ascontiguousarray(wexpT)  # [INNER+1, 36]
    cstf = _const_blobs()

    # Per-core gather row indices [128, 6]: col h*3+ci holds the first row of
    # span chunk ci ([0:4), [4:6), [6:8)) for sample h*128+p, or BIG when the
    # span doesn't reach that chunk (dropped by the DGE bounds check).
    BIG = 100000
    starts = pos[:, :, 0].astype(np.int64)                  # [NCORES, BC]
    lens = (pos[:, :, 1] - pos[:, :, 0]).astype(np.int64)
    base = np.arange(BC, dtype=np.int64) * S
    i0 = base[None, :] + starts
    c1 = np.where(lens > 4, i0 + 4, BIG)
    c2 = np.where(lens > 6, i0 + 6, BIG)
    gidx = np.stack([i0, c1, c2], axis=-1).reshape(NCORES, 2, 128, 3)
    gidx = np.ascontiguousarray(
        gidx.transpose(0, 2, 1, 3).reshape(NCORES, 128, 6).astype(np.int32))

    return [
        {"emb": emb[i], "pos": pos[i], "cat": cat[i], "wbT": wbT, "bb": bb,
         "wexpT": wexpT, "cstf": cstf, "gidx": gidx[i],
         "onesd": np.ones((1, 256), dtype=np.float32)}
        for i in range(NCORES)
    ]


def _run(in_maps, **kw):
    nc = _get_nc()
    return run_bass_kernel_spmd(nc, in_maps, core_ids=list(range(NCORES)), **kw)


def kernel(embeddings, position_indexes, categories, W_base, b_base, W_experts,
           b_experts):
    in_maps = _prep_inputs(embeddings, position_indexes, categories, W_base,
                           b_base, W_experts, b_experts)
    res = _run(in_maps)
    return np.concatenate([r["out"] for r in res.results], axis=0)
